# revision 4
# baseline (speedup 1.0000x reference)
"""Trainium2 Bass kernel for CustomPointScatter.

Reference computation:
    pillar_feat = point_features.mean(axis=1)            # [N, C]
    out = zeros([B, C, H, W]); out[b, :, y, x] = pillar_feat

Sharding: each of the 8 cores owns one output region (b, y_half) of shape
[C, H/2, W].  The host partitions pillars by destination region, pads every
group to a common multiple-of-128 size, and hands each core its pillars plus
per-pillar destination row offsets.  On device the output region is laid out
position-major as [H/2 * W (+pad), C] so each pillar is a single contiguous
256 B row write, done with an indirect (scatter) DMA.  ExternalOutput DRAM is
delivered zero-initialised by the runtime, so only occupied rows are written.
The host reassembles the regions and transposes to [B, C, H, W].
"""

import numpy as np

import concourse.bacc as bacc
import concourse.bass as bass
import concourse.mybir as mybir
import concourse.tile as tile
from concourse.bass_utils import run_bass_kernel_spmd

B, H, W = 4, 512, 512
N_PILLARS, N_POINTS, C = 40000, 32, 64
N_CORES = 8
P = 128
HALF = H // 2            # 256 rows of the BEV grid per core
REGION_ROWS = HALF * W   # 131072 positions per core
PAD_ROWS = P             # dump rows for padded (inactive) pillars
OUT_ROWS = REGION_ROWS + PAD_ROWS


def build_nc(nmax, n_points=N_POINTS, c=C, out_rows=OUT_ROWS):
    """One SPMD program: mean over points + scatter rows into the region."""
    T = nmax // P
    D = n_points * c
    nc = bacc.Bacc("TRN2", target_bir_lowering=False)
    pf = nc.dram_tensor("pf", [nmax, D], mybir.dt.float32, kind="ExternalInput")
    offs = nc.dram_tensor("offs", [P, T], mybir.dt.int32, kind="ExternalInput")
    out = nc.dram_tensor("out", [out_rows, c], mybir.dt.float32, kind="ExternalOutput")
    with tile.TileContext(nc) as tc:
        with (
            tc.tile_pool(name="io", bufs=4) as io_pool,
            tc.tile_pool(name="misc", bufs=1) as misc,
        ):
            offs_sb = misc.tile([P, T], mybir.dt.int32)
            nc.sync.dma_start(out=offs_sb[:], in_=offs[:])
            for t in range(T):
                pf_sb = io_pool.tile([P, D], mybir.dt.float32, tag="pf")
                nc.sync.dma_start(out=pf_sb[:], in_=pf[t * P:(t + 1) * P, :])
                feat = io_pool.tile([P, c], mybir.dt.float32, tag="feat")
                # row layout is [pt0 ch0..chC-1, pt1 ch0.., ...]; reduce the
                # strided point axis, keeping channels
                nc.vector.reduce_sum(
                    out=feat[:],
                    in_=pf_sb[:].rearrange("p (pts ch) -> p ch pts", ch=c),
                    axis=mybir.AxisListType.X,
                )
                nc.scalar.mul(out=feat[:], in_=feat[:], mul=1.0 / n_points)
                nc.gpsimd.indirect_dma_start(
                    out=out[:],
                    out_offset=bass.IndirectOffsetOnAxis(ap=offs_sb[:, t:t + 1], axis=0),
                    in_=feat[:],
                    in_offset=None,
                )
    nc.finalize()  # Bacc.compile(): splits multi-waits for TRN2 codegen
    return nc


def shard_inputs(point_features, voxel_coords):
    pf = np.ascontiguousarray(
        np.asarray(point_features, dtype=np.float32).reshape(N_PILLARS, N_POINTS * C)
    )
    vc = np.asarray(voxel_coords)
    b = vc[:, 0].astype(np.int64)
    y = vc[:, 2].astype(np.int64)
    x = vc[:, 3].astype(np.int64)
    upper = (y >= HALF).astype(np.int64)
    region = b * 2 + upper
    off = (y - upper * HALF) * W + x  # row offset within the owned region
    idx_r = [np.nonzero(region == r)[0] for r in range(N_CORES)]
    nmax = max(len(ix) for ix in idx_r)
    nmax = max(P, ((nmax + P - 1) // P) * P)
    in_maps = []
    for r in range(N_CORES):
        ix = idx_r[r]
        pf_r = np.zeros((nmax, N_POINTS * C), np.float32)
        pf_r[: len(ix)] = pf[ix]
        offs_r = np.full(nmax, REGION_ROWS, np.int32)  # pad rows -> dump row
        offs_r[: len(ix)] = off[ix].astype(np.int32)
        # pillar j = t*128 + p lives at offs_arr[p, t]
        offs_arr = np.ascontiguousarray(offs_r.reshape(-1, P).T)
        in_maps.append({"pf": pf_r, "offs": offs_arr})
    return in_maps, nmax


def assemble(results):
    out = np.empty((B, C, H, W), np.float32)
    for r in range(N_CORES):
        o = results[r]["out"][:REGION_ROWS].reshape(HALF, W, C)
        b_, half = divmod(r, 2)
        out[b_, :, half * HALF:(half + 1) * HALF, :] = o.transpose(2, 0, 1)
    return out


def run(point_features, voxel_coords, trace=False, **spmd_kwargs):
    in_maps, nmax = shard_inputs(point_features, voxel_coords)
    nc = build_nc(nmax)
    br = run_bass_kernel_spmd(
        nc, in_maps, list(range(N_CORES)), trace=trace, **spmd_kwargs
    )
    return assemble(br.results), br


def kernel(point_features, voxel_coords):
    out, _ = run(point_features, voxel_coords)
    return out
